# revision 20
# baseline (speedup 1.0000x reference)
"""Trainium2 Bass kernel for nn_GCBlock (gnn_message_passing).

Data-parallel over batch (2048 -> 8 cores). The gumbel straight-through gate
is numerically an exact one-hot (hard + soft - soft == hard), so samples are
sorted by gate type on the host and each group of NB=6 samples takes one
uniform path:
  t0: H = FC(A1@x)
  t1: H = FC(A1@x) + FC2(x),   FC2 = fc_w @ (adj_t*band)  (folded on host)
  t2: H = FC((A1+A3)@x)
  t3: H = FC(A1@x + x4),       x4 = lo.shift_dn(x) + hi.shift_up(x)
All matmuls bf16 with fp32 PSUM accumulation. Per sample the transpose to the
time-on-partition layout is fused with the joint mix: x-half is the stationary
operand and [AL^T | I66] the moving operand, producing (AL@x)^T and x^T in one
matmul (interleaved 132-wide blocks); the FC reads the two streams back with
strided views. The kernel outputs pre-LN H in transposed layout (bf16); fc_b,
LN, alpha/beta and the f32 residual x + h are applied on the host.
"""
import numpy as np
import ml_dtypes

B, V, T, J = 2048, 66, 256, 22
N_CORES = 8
NB = 6                     # samples per group (3 samples x 132 <= one bank)
FD = NB * V                # 396
BF16 = ml_dtypes.bfloat16

_CACHE = {}


def _build_nc(Gs):
    import contextlib
    import concourse.bacc as bacc
    import concourse.mybir as mybir
    import concourse.tile as tile

    f32 = mybir.dt.float32
    bf16 = mybir.dt.bfloat16
    Alu = mybir.AluOpType
    G = sum(Gs)

    # const blobs: cb0 needed by stage A of every type; cb1 only by t1/t3.
    CB0 = 2 * 132 + 4 * 128          # rhs2 pair + wq
    CB1 = 8 * 128 + 4 * FD           # w2q + sudzs + at3
    xg = nc_dram = None
    nc = bacc.Bacc("TRN2", target_bir_lowering=False, debug=False,
                   num_devices=N_CORES)
    xg = nc.dram_tensor("xg", [G, V, NB * T], bf16, kind="ExternalInput").ap()
    cb0 = nc.dram_tensor("cb0", [128, CB0], bf16, kind="ExternalInput").ap()
    cb1 = nc.dram_tensor("cb1", [128, CB1], bf16, kind="ExternalInput").ap()
    yt = nc.dram_tensor("yt", [G, 128, 2 * FD], bf16, kind="ExternalOutput").ap()

    with tile.TileContext(nc) as tc:
        with contextlib.ExitStack() as ctx:
            cpool = ctx.enter_context(tc.tile_pool(name="consts", bufs=1))
            xpool = ctx.enter_context(tc.tile_pool(name="xin", bufs=3))
            spool = ctx.enter_context(tc.tile_pool(name="work", bufs=2))
            pp = ctx.enter_context(tc.tile_pool(name="ps", bufs=1, space="PSUM"))

            cbt0 = cpool.tile([128, CB0], bf16, name="cbt0", tag="cbt0")
            nc.sync.dma_start(cbt0[:], cb0[:])
            cbt1 = cpool.tile([128, CB1], bf16, name="cbt1", tag="cbt1")
            nc.sync.dma_start(cbt1[:], cb1[:])

            off0 = [0]
            off1 = [0]

            def take(cbt, off, pdim, w):
                v_ = cbt[0:pdim, off[0]:off[0] + w]
                off[0] += w
                return v_

            c_rhs2 = [take(cbt0, off0, V, 132) for _ in range(2)]
            c_wq = [[take(cbt0, off0, 128, 128) for _ in range(2)]
                    for _ in range(2)]          # [kh][F]... filled row-major
            c_w2q = [[take(cbt1, off1, 128, 128) for _ in range(2)]
                     for _ in range(2)]
            c_sud = [take(cbt1, off1, 128, 128) for _ in range(4)]
            # at3[d] laid out (h0|h1) contiguous -> [p, 2, FD] chunk views
            _cat3v = [take(cbt1, off1, 128, 2 * FD).rearrange(
                "p (c w) -> p c w", w=FD) for _ in range(2)]

            # PSUM chunk layout: 2-bank tiles [128, 1024], data chunks at
            # col 0 and col 512 (bank starts). BK = 512.
            BK = 512

            def ch2(tile_ap):
                """[128, 1024] tile -> strided [128, 2, FD] view of chunks."""
                return tile_ap.rearrange("p (c w) -> p c w", w=BK)[:, :, 0:FD]

            def emit_group(g, ty, sxg, xoff, so_ap):
                rv = c_rhs2[1 if ty == 2 else 0]
                fused = ty in (1, 3)
                cp_eng = [nc.scalar.copy, nc.vector.tensor_copy]

                # ---- stage A: fused transpose + joint mix ----
                if fused:
                    # pAB[h]: chunk c holds samples 3c..3c+2 as [x1T|xT]x132
                    pAB = [pp.tile([128, 1024], f32, name="pab",
                                   tag=f"pab{h}") for h in range(2)]
                    for i in range(NB):
                        c, j = i // 3, i % 3
                        for h in range(2):
                            lhs = sxg[:, xoff + i * T + 128 * h:
                                      xoff + i * T + 128 * (h + 1)]
                            nc.tensor.matmul(
                                pAB[h][:, BK * c + 132 * j:
                                       BK * c + 132 * (j + 1)],
                                lhs, rv, start=True, stop=True)
                else:
                    # one tile: chunk 0 = h0, chunk 1 = h1 (alt tag per group)
                    pXA = pp.tile([128, 1024], f32, name="pxa",
                                  tag=f"pab{g % 2}")
                    for i in range(NB):
                        for h in range(2):
                            lhs = sxg[:, xoff + i * T + 128 * h:
                                      xoff + i * T + 128 * (h + 1)]
                            nc.tensor.matmul(
                                pXA[:, BK * h + 66 * i:BK * h + 66 * (i + 1)],
                                lhs, rv[:, 0:66], start=True, stop=True)

                # ---- stage B: one strided evacuation per tile ----
                if fused:
                    sxat = [spool.tile([128, 2 * FD], bf16, name="sxat",
                                       tag=f"sxat{h}") for h in range(2)]
                    for h in range(2):
                        cp_eng[h](sxat[h][:].rearrange(
                            "p (c w) -> p c w", w=FD), ch2(pAB[h][:]))
                    # strided stream views: [p, NB, 0:66]=x1T, [66:132]=xT
                    sxa = [sxat[h][:].rearrange("p (n w) -> p n w", w=132)
                           [:, :, 0:66] for h in range(2)]
                    sxt = [sxat[h][:].rearrange("p (n w) -> p n w", w=132)
                           [:, :, 66:132] for h in range(2)]
                else:
                    sxa_t = spool.tile([128, 2 * FD], bf16, name="sxa",
                                       tag="sxa")
                    cp_eng[g % 2](sxa_t[:].rearrange("p (c w) -> p c w", w=FD),
                                  ch2(pXA[:]))
                    sxa = [sxa_t[:, 0:FD], sxa_t[:, FD:2 * FD]]
                    sxt = None

                # ---- stage C/D: per-node banded term (type 3) ----
                if ty == 3:
                    # pSL chunks: c=h; reuses the pab slots
                    pSL = pp.tile([128, 1024], f32, name="psl", tag="pab0")
                    pSR = pp.tile([128, 1024], f32, name="psr", tag="pab1")
                    # SL[t] = x[t-1]; SR[t] = x[t+1]  (cross-half seams)
                    nc.tensor.matmul(pSL[:, 0:FD], c_sud[0], sxt[0],
                                     start=True, stop=True)
                    nc.tensor.matmul(pSL[:, BK:BK + FD], c_sud[0], sxt[1],
                                     start=True, stop=False)
                    nc.tensor.matmul(pSL[:, BK:BK + FD], c_sud[2], sxt[0],
                                     start=False, stop=True)
                    nc.tensor.matmul(pSR[:, BK:BK + FD], c_sud[1], sxt[1],
                                     start=True, stop=True)
                    nc.tensor.matmul(pSR[:, 0:FD], c_sud[1], sxt[0],
                                     start=True, stop=False)
                    nc.tensor.matmul(pSR[:, 0:FD], c_sud[3], sxt[1],
                                     start=False, stop=True)
                    w3 = spool.tile([128, 2 * FD], bf16, name="w3", tag="w3")
                    w4 = spool.tile([128, 2 * FD], bf16, name="w4", tag="w4")
                    x4t = spool.tile([128, 2 * FD], bf16, name="x4t",
                                     tag="x4t")
                    nc.vector.tensor_tensor(
                        w3[:].rearrange("p (c w) -> p c w", w=FD),
                        ch2(pSL[:]), _cat3v[0], Alu.mult)
                    nc.vector.tensor_tensor(
                        w4[:].rearrange("p (c w) -> p c w", w=FD),
                        ch2(pSR[:]), _cat3v[1], Alu.mult)
                    nc.gpsimd.tensor_tensor(x4t[:], w3[:], w4[:], Alu.add)
                    x4s = [x4t[:, 0:FD], x4t[:, FD:2 * FD]]

                # ---- stage E: temporal FC, PSUM-accumulated streams ----
                pH = pp.tile([128, 1024], f32, name="ph", tag="ph", bufs=2)
                if ty == 1:
                    streams = [(c_wq, sxa), (c_w2q, sxt)]
                elif ty == 3:
                    streams = [(c_wq, sxa), (c_wq, x4s)]
                else:
                    streams = [(c_wq, sxa)]
                ns = len(streams)
                for F in range(2):
                    for si, (w, s) in enumerate(streams):
                        for kh in range(2):
                            nc.tensor.matmul(
                                pH[:, BK * F:BK * F + FD], w[kh][F], s[kh],
                                start=(si == 0 and kh == 0),
                                stop=(si == ns - 1 and kh == 1))

                # ---- stage F: one strided out-cast ----
                cp_eng[(g + 1) % 2](so_ap.rearrange("p (c w) -> p c w", w=FD),
                                    ch2(pH[:]))

            QG = 4                 # groups per input DMA
            OG = 2                 # groups per output DMA
            g = 0
            for ty in (0, 2, 1, 3):
                ngroups = Gs[ty]
                gi = 0
                while gi < ngroups:
                    nq = min(2 if g == 0 else QG, ngroups - gi)
                    sxg = xpool.tile([V, QG * NB * T], bf16, name="sxg",
                                     tag="sxg")
                    nc.gpsimd.dma_start(
                        sxg[:, 0:nq * NB * T].rearrange(
                            "v (g t) -> v g t", g=nq),
                        xg[g:g + nq].rearrange("g v t -> v g t"))
                    k = 0
                    while k < nq:
                        no = min(OG, nq - k)
                        so = spool.tile([128, OG * 2 * FD], bf16, name="so",
                                        tag="so", bufs=3)
                        for j in range(no):
                            emit_group(g, ty, sxg, (k + j) * NB * T,
                                       so[:, j * 2 * FD:(j + 1) * 2 * FD])
                            g += 1
                        nc.sync.dma_start(
                            yt[g - no:g].rearrange("g p w -> p g w"),
                            so[:, 0:no * 2 * FD].rearrange(
                                "p (g w) -> p g w", g=no))
                        k += no
                    gi += nq

    nc.compile()
    return nc


def _gate_types(x, mlp, if_make_dynamic, tau):
    """Exact replication of the reference gating; forward value is one-hot."""
    import jax
    import jax.numpy as jnp

    if not if_make_dynamic:
        return np.zeros(x.shape[0], dtype=np.int64)
    prob = jnp.asarray(x).mean(axis=1) @ jnp.asarray(mlp)
    u = jax.random.uniform(jax.random.key(42), prob.shape,
                           minval=1e-10, maxval=1.0)
    gumbel = -jnp.log(-jnp.log(u))
    soft = jax.nn.softmax((prob + gumbel) / tau, axis=-1)
    return np.asarray(jnp.argmax(soft, axis=-1), dtype=np.int64)


def kernel(x, mlp, adj_j, adj_t, adj_jc, adj_tj, fc_w, fc_b, alpha, beta,
           if_make_dynamic, tau):
    from concourse.bass_utils import run_bass_kernel_spmd

    x = np.asarray(x, dtype=np.float32)
    mlp = np.asarray(mlp, dtype=np.float32)
    adj_j = np.asarray(adj_j, dtype=np.float32)
    adj_t = np.asarray(adj_t, dtype=np.float32)
    adj_jc = np.asarray(adj_jc, dtype=np.float32)
    adj_tj = np.asarray(adj_tj, dtype=np.float32)
    fc_w = np.asarray(fc_w, dtype=np.float32)
    fc_b = np.asarray(fc_b, dtype=np.float32)
    alpha = np.asarray(alpha, dtype=np.float32).reshape(1, V, 1)
    beta = np.asarray(beta, dtype=np.float32).reshape(1, V, 1)

    types = _gate_types(x, mlp, if_make_dynamic, tau)
    counts = np.bincount(types, minlength=4)
    percore = N_CORES * NB
    Gs = tuple(int(np.ceil(c / percore)) for c in counts)
    G = sum(Gs)
    BLp = NB * G

    # per-core sample assignment: type-sorted, padded to uniform group counts
    order = np.argsort(types, kind="stable")
    perm = np.zeros((N_CORES, BLp), np.int64)
    real = np.zeros((N_CORES, BLp), bool)
    off = 0
    # group order in the program is (0, 2, 1, 3); slots must match
    slot_of_type = {}
    slot = 0
    for t in (0, 2, 1, 3):
        slot_of_type[t] = slot
        slot += NB * Gs[t]
    for t in range(4):
        n = int(counts[t])
        cap = NB * Gs[t]
        idx = order[off:off + n]
        off += n
        padded = np.zeros(N_CORES * cap, np.int64)
        padded[:n] = idx
        if N_CORES * cap > n and n > 0:
            padded[n:] = idx[0]
        rm = np.zeros(N_CORES * cap, bool)
        rm[:n] = True
        s = slot_of_type[t]
        perm[:, s:s + cap] = padded.reshape(N_CORES, cap)
        real[:, s:s + cap] = rm.reshape(N_CORES, cap)

    # ---- host-folded constants (two packed blobs) ----
    A1 = np.kron(adj_j, np.eye(3, dtype=np.float32))
    A3 = np.zeros((V, V), np.float32)
    for j in range(J):
        A3[3 * j:3 * j + 3, 3 * j:3 * j + 3] = adj_jc[j]
    I66 = np.eye(V, dtype=np.float32)

    idxs = np.arange(T)
    bandm = (np.abs(idxs[:, None] - idxs[None, :]) == 1).astype(np.float32)
    W2 = fc_w @ (adj_t * bandm)

    atj_lo = np.zeros((V, T), np.float32)
    atj_hi = np.zeros((V, T), np.float32)
    atj_lo[:, 1:] = adj_tj[:, np.arange(1, T), np.arange(0, T - 1)]
    atj_hi[:, :-1] = adj_tj[:, np.arange(0, T - 1), np.arange(1, T)]

    CB0 = 2 * 132 + 4 * 128
    CB1 = 8 * 128 + 4 * FD
    cb0 = np.zeros((128, CB0), np.float32)
    cb1 = np.zeros((128, CB1), np.float32)
    col = [0]

    def put(dst, arr):
        p, w = arr.shape
        dst[0:p, col[0]:col[0] + w] = arr
        col[0] += w

    put(cb0, np.concatenate([A1.T, I66], axis=1))
    put(cb0, np.concatenate([(A1 + A3).T, I66], axis=1))
    for kh in range(2):
        for F in range(2):
            put(cb0, fc_w[128 * F:128 * (F + 1), 128 * kh:128 * (kh + 1)].T)
    assert col[0] == CB0
    col[0] = 0
    for kh in range(2):
        for F in range(2):
            put(cb1, W2[128 * F:128 * (F + 1), 128 * kh:128 * (kh + 1)].T)
    sud0 = np.eye(128, k=1, dtype=np.float32)
    sud1 = np.eye(128, k=-1, dtype=np.float32)
    zs0 = np.zeros((128, 128), np.float32)
    zs0[127, 0] = 1.0               # SL h1 row0 = x[127] (from h0)
    zs1 = np.zeros((128, 128), np.float32)
    zs1[0, 127] = 1.0               # SR h0 row127 = x[128] (from h1)
    put(cb1, sud0)
    put(cb1, sud1)
    put(cb1, zs0)
    put(cb1, zs1)
    for src in (atj_lo, atj_hi):
        for h in range(2):
            put(cb1, np.tile(src[:, 128 * h:128 * (h + 1)].T, (1, NB)))
    assert col[0] == CB1
    cb0 = cb0.astype(BF16)
    cb1 = cb1.astype(BF16)

    x_bf = x.astype(BF16)
    in_maps = []
    for c in range(N_CORES):
        xp = x_bf[perm[c]]                                     # [BLp, V, T]
        xgc = np.ascontiguousarray(
            xp.reshape(G, NB, V, T).transpose(0, 2, 1, 3)
        ).reshape(G, V, NB * T)
        in_maps.append(dict(xg=xgc, cb0=cb0, cb1=cb1))

    # program group order is types (0, 2, 1, 3); Gs passed in type order but
    # _build_nc iterates (0, 2, 1, 3) so slot layout matches perm layout.
    if Gs not in _CACHE:
        _CACHE[Gs] = _build_nc(Gs)
    nc = _CACHE[Gs]
    res = run_bass_kernel_spmd(nc, in_maps, core_ids=list(range(N_CORES)),
                               **_RUN_KW)
    _LAST_RES.clear()
    _LAST_RES["res"] = res

    # ---- host epilogue: un-transpose, + fc_b, LN, alpha/beta, residual ----
    out = np.empty((B, V, T), dtype=np.float32)
    for c in range(N_CORES):
        H = np.asarray(res.results[c]["yt"], dtype=np.float32)
        # yt: [G, 128(p), 2(F), NB, V] -> H[b, f=F*128+p, v]
        H = (H.reshape(G, 128, 2, NB, V).transpose(0, 3, 2, 1, 4)
             .reshape(BLp, T, V).transpose(0, 2, 1))           # [b, V, T(f)]
        H += fc_b[None, None, :]
        m = H.mean(axis=1, keepdims=True)
        var = ((H - m) ** 2).mean(axis=1, keepdims=True)
        h = (H - m) / np.sqrt(var + 1e-5) * alpha + beta
        res_c = x[perm[c]] + h
        msk = real[c]
        out[perm[c][msk]] = res_c[msk]
    return out


_RUN_KW = {}
_LAST_RES = {}


# revision 25
# speedup vs baseline: 1.0582x; 1.0582x over previous
"""Trainium2 Bass kernel for nn_GCBlock (gnn_message_passing).

Data-parallel over batch (2048 -> 8 cores). The gumbel straight-through gate
is numerically an exact one-hot (hard + soft - soft == hard), so samples are
sorted by gate type on the host and each group of NB=6 samples takes one
uniform path:
  t0: H = FC(A1@x)
  t1: H = FC(A1@x) + FC2(x),   FC2 = fc_w @ (adj_t*band)  (folded on host)
  t2: H = FC((A1+A3)@x)
  t3: H = FC(A1@x + x4),       x4 = lo.shift_dn(x) + hi.shift_up(x)
All matmuls bf16 with fp32 PSUM accumulation. Per sample the transpose to the
time-on-partition layout is fused with the joint mix: x-half is the stationary
operand and [AL^T | I66] the moving operand, producing (AL@x)^T and x^T in one
matmul (interleaved 132-wide blocks); the FC reads the two streams back with
strided views. The kernel outputs pre-LN H in transposed layout (bf16); fc_b,
LN, alpha/beta and the f32 residual x + h are applied on the host.
"""
import numpy as np
import ml_dtypes

B, V, T, J = 2048, 66, 256, 22
N_CORES = 8
NB = 6                     # samples per group (3 samples x 132 <= one bank)
FD = NB * V                # 396
BF16 = ml_dtypes.bfloat16

_CACHE = {}


def _build_nc(Gs):
    import contextlib
    import concourse.bacc as bacc
    import concourse.mybir as mybir
    import concourse.tile as tile

    f32 = mybir.dt.float32
    bf16 = mybir.dt.bfloat16
    Alu = mybir.AluOpType
    G = sum(Gs)

    # const blobs: cb0 needed by stage A of every type; cb1 only by t1/t3.
    CB0 = 2 * 132 + 4 * 128          # rhs2 pair + wq
    CB1 = 8 * 128 + 4 * FD           # w2q + sudzs + at3
    xg = nc_dram = None
    nc = bacc.Bacc("TRN2", target_bir_lowering=False, debug=False,
                   num_devices=N_CORES)
    xg = nc.dram_tensor("xg", [G, V, NB * T], bf16, kind="ExternalInput").ap()
    cb0 = nc.dram_tensor("cb0", [128, CB0], bf16, kind="ExternalInput").ap()
    cb1 = nc.dram_tensor("cb1", [128, CB1], bf16, kind="ExternalInput").ap()
    yt = nc.dram_tensor("yt", [G, 128, 2 * FD], bf16, kind="ExternalOutput").ap()

    with tile.TileContext(nc) as tc:
        with contextlib.ExitStack() as ctx:
            cpool = ctx.enter_context(tc.tile_pool(name="consts", bufs=1))
            xpool = ctx.enter_context(tc.tile_pool(name="xin", bufs=3))
            spool = ctx.enter_context(tc.tile_pool(name="work", bufs=3))
            pp = ctx.enter_context(tc.tile_pool(name="ps", bufs=1, space="PSUM"))

            cbt0 = cpool.tile([128, CB0], bf16, name="cbt0", tag="cbt0")
            nc.sync.dma_start(cbt0[:], cb0[:])
            cbt1 = cpool.tile([128, CB1], bf16, name="cbt1", tag="cbt1")
            nc.sync.dma_start(cbt1[:], cb1[:])

            off0 = [0]
            off1 = [0]

            def take(cbt, off, pdim, w):
                v_ = cbt[0:pdim, off[0]:off[0] + w]
                off[0] += w
                return v_

            c_rhs2 = [take(cbt0, off0, V, 132) for _ in range(2)]
            c_wq = [[take(cbt0, off0, 128, 128) for _ in range(2)]
                    for _ in range(2)]          # [kh][F]... filled row-major
            c_w2q = [[take(cbt1, off1, 128, 128) for _ in range(2)]
                     for _ in range(2)]
            c_sud = [take(cbt1, off1, 128, 128) for _ in range(4)]
            c_at3 = [[take(cbt1, off1, 128, FD) for _ in range(2)]
                     for _ in range(2)]

            def emit_group(g, ty, sxg, xoff, so_ap):
                rv = c_rhs2[1 if ty == 2 else 0]
                fused = ty in (1, 3)

                # ---- stage A: fused transpose + joint mix ----
                if fused:
                    # interleaved [x1T | xT] 132-wide blocks, 3 samples/bank
                    pAB = [[pp.tile([128, FD], f32, name="pab",
                                    tag=f"pa{h}{c}") for c in range(2)]
                           for h in range(2)]
                    for i in range(NB):
                        c, j = i // 3, i % 3
                        for h in range(2):
                            lhs = sxg[:, xoff + i * T + 128 * h:
                                      xoff + i * T + 128 * (h + 1)]
                            nc.tensor.matmul(
                                pAB[h][c][:, 132 * j:132 * (j + 1)],
                                lhs, rv, start=True, stop=True)
                else:
                    pXA = [pp.tile([128, FD], f32, name="pxa",
                                   tag=f"pa{h}{g % 2}") for h in range(2)]
                    for i in range(NB):
                        for h in range(2):
                            lhs = sxg[:, xoff + i * T + 128 * h:
                                      xoff + i * T + 128 * (h + 1)]
                            nc.tensor.matmul(pXA[h][:, 66 * i:66 * (i + 1)],
                                             lhs, rv[:, 0:66],
                                             start=True, stop=True)

                # ---- stage B: evacuate to SBUF bf16 (ACT/DVE split) ----
                if fused:
                    sxat = [spool.tile([128, 2 * FD], bf16, name="sxat",
                                       tag=f"sxat{h}") for h in range(2)]
                    for h in range(2):
                        eng = [nc.scalar.copy, nc.vector.tensor_copy]
                        eng[h](sxat[h][:, 0:FD], pAB[h][0][:])
                        eng[1 - h](sxat[h][:, FD:2 * FD], pAB[h][1][:])
                    # strided stream views: [p, NB, 0:66]=x1T, [66:132]=xT
                    sxa = [sxat[h][:].rearrange("p (n w) -> p n w", w=132)
                           [:, :, 0:66] for h in range(2)]
                    sxt = [sxat[h][:].rearrange("p (n w) -> p n w", w=132)
                           [:, :, 66:132] for h in range(2)]
                else:
                    sxa_t = [spool.tile([128, FD], bf16, name="sxa",
                                        tag=f"sxa{h}") for h in range(2)]
                    nc.scalar.copy(sxa_t[0][:], pXA[0][:])
                    nc.vector.tensor_copy(sxa_t[1][:], pXA[1][:])
                    sxa = [sxa_t[h][:] for h in range(2)]
                    sxt = None

                # ---- stage C/D: per-node banded term (type 3) ----
                if ty == 3:
                    pSL = [pp.tile([128, FD], f32, name="psl", tag=f"pa{h}0")
                           for h in range(2)]
                    pSR = [pp.tile([128, FD], f32, name="psr", tag=f"pa{h}1")
                           for h in range(2)]
                    # SL[t] = x[t-1]; SR[t] = x[t+1]  (cross-half seams)
                    nc.tensor.matmul(pSL[0][:], c_sud[0], sxt[0],
                                     start=True, stop=True)
                    nc.tensor.matmul(pSL[1][:], c_sud[0], sxt[1],
                                     start=True, stop=False)
                    nc.tensor.matmul(pSL[1][:], c_sud[2], sxt[0],
                                     start=False, stop=True)
                    nc.tensor.matmul(pSR[1][:], c_sud[1], sxt[1],
                                     start=True, stop=True)
                    nc.tensor.matmul(pSR[0][:], c_sud[1], sxt[0],
                                     start=True, stop=False)
                    nc.tensor.matmul(pSR[0][:], c_sud[3], sxt[1],
                                     start=False, stop=True)
                    # w3/w4 halves per h; one combined gpsimd add
                    w3 = spool.tile([128, 2 * FD], bf16, name="w3", tag="w3")
                    w4 = spool.tile([128, 2 * FD], bf16, name="w4", tag="w4")
                    x4t = spool.tile([128, 2 * FD], bf16, name="x4t",
                                     tag="x4t")
                    for h in range(2):
                        nc.vector.tensor_tensor(w3[:, h * FD:(h + 1) * FD],
                                                pSL[h][:], c_at3[0][h],
                                                Alu.mult)
                        nc.vector.tensor_tensor(w4[:, h * FD:(h + 1) * FD],
                                                pSR[h][:], c_at3[1][h],
                                                Alu.mult)
                    nc.gpsimd.tensor_tensor(x4t[:], w3[:], w4[:], Alu.add)
                    x4s = [x4t[:, 0:FD], x4t[:, FD:2 * FD]]

                # ---- stage E: temporal FC, PSUM-accumulated streams ----
                pH = [pp.tile([128, FD], f32, name="ph", tag=f"ph{F}", bufs=2)
                      for F in range(2)]
                if ty == 1:
                    streams = [(c_wq, sxa), (c_w2q, sxt)]
                elif ty == 3:
                    streams = [(c_wq, sxa), (c_wq, x4s)]
                else:
                    streams = [(c_wq, sxa)]
                ns = len(streams)
                for F in range(2):
                    for si, (w, s) in enumerate(streams):
                        for kh in range(2):
                            nc.tensor.matmul(
                                pH[F][:], w[kh][F], s[kh],
                                start=(si == 0 and kh == 0),
                                stop=(si == ns - 1 and kh == 1))

                # ---- stage F: out copies (ACT/DVE split) ----
                nc.scalar.copy(so_ap[:, 0:FD], pH[0][:])
                nc.vector.tensor_copy(so_ap[:, FD:2 * FD], pH[1][:])

            QG = 4                 # groups per input DMA
            OG = 4                 # groups per output DMA
            g = 0
            for ty in (0, 2, 1, 3):
                ngroups = Gs[ty]
                gi = 0
                while gi < ngroups:
                    nq = min(2 if g == 0 else QG, ngroups - gi)
                    sxg = xpool.tile([V, QG * NB * T], bf16, name="sxg",
                                     tag="sxg")
                    nc.gpsimd.dma_start(
                        sxg[:, 0:nq * NB * T].rearrange(
                            "v (g t) -> v g t", g=nq),
                        xg[g:g + nq].rearrange("g v t -> v g t"))
                    k = 0
                    while k < nq:
                        no = min(OG, nq - k)
                        so = spool.tile([128, OG * 2 * FD], bf16, name="so",
                                        tag="so", bufs=3)
                        for j in range(no):
                            emit_group(g, ty, sxg, (k + j) * NB * T,
                                       so[:, j * 2 * FD:(j + 1) * 2 * FD])
                            g += 1
                        nc.sync.dma_start(
                            yt[g - no:g].rearrange("g p w -> p g w"),
                            so[:, 0:no * 2 * FD].rearrange(
                                "p (g w) -> p g w", g=no))
                        k += no
                    gi += nq

    nc.compile()
    return nc


def _gate_types(x, mlp, if_make_dynamic, tau):
    """Exact replication of the reference gating; forward value is one-hot."""
    import jax
    import jax.numpy as jnp

    if not if_make_dynamic:
        return np.zeros(x.shape[0], dtype=np.int64)
    prob = jnp.asarray(x).mean(axis=1) @ jnp.asarray(mlp)
    u = jax.random.uniform(jax.random.key(42), prob.shape,
                           minval=1e-10, maxval=1.0)
    gumbel = -jnp.log(-jnp.log(u))
    soft = jax.nn.softmax((prob + gumbel) / tau, axis=-1)
    return np.asarray(jnp.argmax(soft, axis=-1), dtype=np.int64)


def kernel(x, mlp, adj_j, adj_t, adj_jc, adj_tj, fc_w, fc_b, alpha, beta,
           if_make_dynamic, tau):
    from concourse.bass_utils import run_bass_kernel_spmd

    x = np.asarray(x, dtype=np.float32)
    mlp = np.asarray(mlp, dtype=np.float32)
    adj_j = np.asarray(adj_j, dtype=np.float32)
    adj_t = np.asarray(adj_t, dtype=np.float32)
    adj_jc = np.asarray(adj_jc, dtype=np.float32)
    adj_tj = np.asarray(adj_tj, dtype=np.float32)
    fc_w = np.asarray(fc_w, dtype=np.float32)
    fc_b = np.asarray(fc_b, dtype=np.float32)
    alpha = np.asarray(alpha, dtype=np.float32).reshape(1, V, 1)
    beta = np.asarray(beta, dtype=np.float32).reshape(1, V, 1)

    types = _gate_types(x, mlp, if_make_dynamic, tau)
    counts = np.bincount(types, minlength=4)
    percore = N_CORES * NB
    Gs = tuple(int(np.ceil(c / percore)) for c in counts)
    G = sum(Gs)
    BLp = NB * G

    # per-core sample assignment: type-sorted, padded to uniform group counts
    order = np.argsort(types, kind="stable")
    perm = np.zeros((N_CORES, BLp), np.int64)
    real = np.zeros((N_CORES, BLp), bool)
    off = 0
    # group order in the program is (0, 2, 1, 3); slots must match
    slot_of_type = {}
    slot = 0
    for t in (0, 2, 1, 3):
        slot_of_type[t] = slot
        slot += NB * Gs[t]
    for t in range(4):
        n = int(counts[t])
        cap = NB * Gs[t]
        idx = order[off:off + n]
        off += n
        padded = np.zeros(N_CORES * cap, np.int64)
        padded[:n] = idx
        if N_CORES * cap > n and n > 0:
            padded[n:] = idx[0]
        rm = np.zeros(N_CORES * cap, bool)
        rm[:n] = True
        s = slot_of_type[t]
        perm[:, s:s + cap] = padded.reshape(N_CORES, cap)
        real[:, s:s + cap] = rm.reshape(N_CORES, cap)

    # ---- host-folded constants (two packed blobs) ----
    A1 = np.kron(adj_j, np.eye(3, dtype=np.float32))
    A3 = np.zeros((V, V), np.float32)
    for j in range(J):
        A3[3 * j:3 * j + 3, 3 * j:3 * j + 3] = adj_jc[j]
    I66 = np.eye(V, dtype=np.float32)

    idxs = np.arange(T)
    bandm = (np.abs(idxs[:, None] - idxs[None, :]) == 1).astype(np.float32)
    W2 = fc_w @ (adj_t * bandm)

    atj_lo = np.zeros((V, T), np.float32)
    atj_hi = np.zeros((V, T), np.float32)
    atj_lo[:, 1:] = adj_tj[:, np.arange(1, T), np.arange(0, T - 1)]
    atj_hi[:, :-1] = adj_tj[:, np.arange(0, T - 1), np.arange(1, T)]

    CB0 = 2 * 132 + 4 * 128
    CB1 = 8 * 128 + 4 * FD
    cb0 = np.zeros((128, CB0), np.float32)
    cb1 = np.zeros((128, CB1), np.float32)
    col = [0]

    def put(dst, arr):
        p, w = arr.shape
        dst[0:p, col[0]:col[0] + w] = arr
        col[0] += w

    put(cb0, np.concatenate([A1.T, I66], axis=1))
    put(cb0, np.concatenate([(A1 + A3).T, I66], axis=1))
    for kh in range(2):
        for F in range(2):
            put(cb0, fc_w[128 * F:128 * (F + 1), 128 * kh:128 * (kh + 1)].T)
    assert col[0] == CB0
    col[0] = 0
    for kh in range(2):
        for F in range(2):
            put(cb1, W2[128 * F:128 * (F + 1), 128 * kh:128 * (kh + 1)].T)
    sud0 = np.eye(128, k=1, dtype=np.float32)
    sud1 = np.eye(128, k=-1, dtype=np.float32)
    zs0 = np.zeros((128, 128), np.float32)
    zs0[127, 0] = 1.0               # SL h1 row0 = x[127] (from h0)
    zs1 = np.zeros((128, 128), np.float32)
    zs1[0, 127] = 1.0               # SR h0 row127 = x[128] (from h1)
    put(cb1, sud0)
    put(cb1, sud1)
    put(cb1, zs0)
    put(cb1, zs1)
    for src in (atj_lo, atj_hi):
        for h in range(2):
            put(cb1, np.tile(src[:, 128 * h:128 * (h + 1)].T, (1, NB)))
    assert col[0] == CB1
    cb0 = cb0.astype(BF16)
    cb1 = cb1.astype(BF16)

    x_bf = x.astype(BF16)
    in_maps = []
    for c in range(N_CORES):
        xp = x_bf[perm[c]]                                     # [BLp, V, T]
        xgc = np.ascontiguousarray(
            xp.reshape(G, NB, V, T).transpose(0, 2, 1, 3)
        ).reshape(G, V, NB * T)
        in_maps.append(dict(xg=xgc, cb0=cb0, cb1=cb1))

    # program group order is types (0, 2, 1, 3); Gs passed in type order but
    # _build_nc iterates (0, 2, 1, 3) so slot layout matches perm layout.
    if Gs not in _CACHE:
        _CACHE[Gs] = _build_nc(Gs)
    nc = _CACHE[Gs]
    res = run_bass_kernel_spmd(nc, in_maps, core_ids=list(range(N_CORES)),
                               **_RUN_KW)
    _LAST_RES.clear()
    _LAST_RES["res"] = res

    # ---- host epilogue: un-transpose, + fc_b, LN, alpha/beta, residual ----
    out = np.empty((B, V, T), dtype=np.float32)
    for c in range(N_CORES):
        H = np.asarray(res.results[c]["yt"], dtype=np.float32)
        # yt: [G, 128(p), 2(F), NB, V] -> H[b, f=F*128+p, v]
        H = (H.reshape(G, 128, 2, NB, V).transpose(0, 3, 2, 1, 4)
             .reshape(BLp, T, V).transpose(0, 2, 1))           # [b, V, T(f)]
        H += fc_b[None, None, :]
        m = H.mean(axis=1, keepdims=True)
        var = ((H - m) ** 2).mean(axis=1, keepdims=True)
        h = (H - m) / np.sqrt(var + 1e-5) * alpha + beta
        res_c = x[perm[c]] + h
        msk = real[c]
        out[perm[c][msk]] = res_c[msk]
    return out


_RUN_KW = {}
_LAST_RES = {}


# revision 26
# speedup vs baseline: 1.2284x; 1.1609x over previous
"""Trainium2 Bass kernel for nn_GCBlock (gnn_message_passing).

Data-parallel over batch (2048 -> 8 cores). The gumbel straight-through gate
is numerically an exact one-hot (hard + soft - soft == hard), so samples are
sorted by gate type on the host and each group of NB=6 samples takes one
uniform path:
  t0: H = FC(A1@x)
  t1: H = FC(A1@x) + FC2(x),   FC2 = fc_w @ (adj_t*band)  (folded on host)
  t2: H = FC((A1+A3)@x)
  t3: H = FC(A1@x + x4),       x4 = lo.shift_dn(x) + hi.shift_up(x)
All matmuls bf16 with fp32 PSUM accumulation. Per sample the transpose to the
time-on-partition layout is fused with the joint mix: x-half is the stationary
operand and [AL^T | I66] the moving operand, producing (AL@x)^T and x^T in one
matmul (interleaved 132-wide blocks); the FC reads the two streams back with
strided views. The kernel outputs pre-LN H in transposed layout (bf16); fc_b,
LN, alpha/beta and the f32 residual x + h are applied on the host.
"""
import numpy as np
import ml_dtypes

B, V, T, J = 2048, 66, 256, 22
N_CORES = 8
NB = 6                     # samples per group (3 samples x 132 <= one bank)
FD = NB * V                # 396
BF16 = ml_dtypes.bfloat16

_CACHE = {}


def _build_nc(Gs):
    import contextlib
    import concourse.bacc as bacc
    import concourse.mybir as mybir
    import concourse.tile as tile

    f32 = mybir.dt.float32
    bf16 = mybir.dt.bfloat16
    Alu = mybir.AluOpType
    G = sum(Gs)

    # const blobs: cb0 needed by stage A of every type; cb1 only by t1/t3.
    CB0 = 2 * 132 + 4 * 128          # rhs2 pair + wq
    CB1 = 8 * 128 + 4 * FD           # w2q + sudzs + at3
    xg = nc_dram = None
    nc = bacc.Bacc("TRN2", target_bir_lowering=False, debug=False,
                   num_devices=N_CORES)
    xg = nc.dram_tensor("xg", [G, V, NB * T], bf16, kind="ExternalInput").ap()
    cb0 = nc.dram_tensor("cb0", [128, CB0], bf16, kind="ExternalInput").ap()
    cb1 = nc.dram_tensor("cb1", [128, CB1], bf16, kind="ExternalInput").ap()
    yt = nc.dram_tensor("yt", [G, 128, 2 * FD], bf16, kind="ExternalOutput").ap()

    with tile.TileContext(nc) as tc:
        with contextlib.ExitStack() as ctx:
            cpool = ctx.enter_context(tc.tile_pool(name="consts", bufs=1))
            xpool = ctx.enter_context(tc.tile_pool(name="xin", bufs=3))
            spool = ctx.enter_context(tc.tile_pool(name="work", bufs=2))
            pp = ctx.enter_context(tc.tile_pool(name="ps", bufs=1, space="PSUM"))

            cbt0 = cpool.tile([128, CB0], bf16, name="cbt0", tag="cbt0")
            nc.sync.dma_start(cbt0[:], cb0[:])
            cbt1 = cpool.tile([128, CB1], bf16, name="cbt1", tag="cbt1")
            nc.sync.dma_start(cbt1[:], cb1[:])

            off0 = [0]
            off1 = [0]

            def take(cbt, off, pdim, w):
                v_ = cbt[0:pdim, off[0]:off[0] + w]
                off[0] += w
                return v_

            c_rhs2 = [take(cbt0, off0, V, 132) for _ in range(2)]
            c_wq = [[take(cbt0, off0, 128, 128) for _ in range(2)]
                    for _ in range(2)]          # [kh][F]... filled row-major
            c_w2q = [[take(cbt1, off1, 128, 128) for _ in range(2)]
                     for _ in range(2)]
            c_sud = [take(cbt1, off1, 128, 128) for _ in range(4)]
            c_at3 = [[take(cbt1, off1, 128, FD) for _ in range(2)]
                     for _ in range(2)]

            def emit_group(g, ty, sxg, xoff, so_ap):
                rv = c_rhs2[1 if ty == 2 else 0]
                fused = ty in (1, 3)

                # ---- stage A: fused transpose + joint mix ----
                if fused:
                    # interleaved [x1T | xT] 132-wide blocks, 3 samples/bank
                    pAB = [[pp.tile([128, FD], f32, name="pab",
                                    tag=f"pa{h}{c}") for c in range(2)]
                           for h in range(2)]
                    for i in range(NB):
                        c, j = i // 3, i % 3
                        for h in range(2):
                            lhs = sxg[:, xoff + i * T + 128 * h:
                                      xoff + i * T + 128 * (h + 1)]
                            nc.tensor.matmul(
                                pAB[h][c][:, 132 * j:132 * (j + 1)],
                                lhs, rv, start=True, stop=True)
                else:
                    pXA = [pp.tile([128, FD], f32, name="pxa",
                                   tag=f"pa{h}{g % 2}") for h in range(2)]
                    for i in range(NB):
                        for h in range(2):
                            lhs = sxg[:, xoff + i * T + 128 * h:
                                      xoff + i * T + 128 * (h + 1)]
                            nc.tensor.matmul(pXA[h][:, 66 * i:66 * (i + 1)],
                                             lhs, rv[:, 0:66],
                                             start=True, stop=True)

                # ---- stage B: evacuate to SBUF bf16 (ACT/DVE split) ----
                if fused:
                    sxat = [spool.tile([128, 2 * FD], bf16, name="sxat",
                                       tag=f"sxat{h}") for h in range(2)]
                    for h in range(2):
                        eng = [nc.scalar.copy, nc.vector.tensor_copy]
                        eng[h](sxat[h][:, 0:FD], pAB[h][0][:])
                        eng[1 - h](sxat[h][:, FD:2 * FD], pAB[h][1][:])
                    # strided stream views: [p, NB, 0:66]=x1T, [66:132]=xT
                    sxa = [sxat[h][:].rearrange("p (n w) -> p n w", w=132)
                           [:, :, 0:66] for h in range(2)]
                    sxt = [sxat[h][:].rearrange("p (n w) -> p n w", w=132)
                           [:, :, 66:132] for h in range(2)]
                else:
                    sxa_t = [spool.tile([128, FD], bf16, name="sxa",
                                        tag=f"sxa{h}") for h in range(2)]
                    nc.scalar.copy(sxa_t[0][:], pXA[0][:])
                    nc.vector.tensor_copy(sxa_t[1][:], pXA[1][:])
                    sxa = [sxa_t[h][:] for h in range(2)]
                    sxt = None

                # ---- stage C/D: per-node banded term (type 3) ----
                if ty == 3:
                    pSL = [pp.tile([128, FD], f32, name="psl", tag=f"pa{h}0")
                           for h in range(2)]
                    pSR = [pp.tile([128, FD], f32, name="psr", tag=f"pa{h}1")
                           for h in range(2)]
                    # SL[t] = x[t-1]; SR[t] = x[t+1]  (cross-half seams)
                    nc.tensor.matmul(pSL[0][:], c_sud[0], sxt[0],
                                     start=True, stop=True)
                    nc.tensor.matmul(pSL[1][:], c_sud[0], sxt[1],
                                     start=True, stop=False)
                    nc.tensor.matmul(pSL[1][:], c_sud[2], sxt[0],
                                     start=False, stop=True)
                    nc.tensor.matmul(pSR[1][:], c_sud[1], sxt[1],
                                     start=True, stop=True)
                    nc.tensor.matmul(pSR[0][:], c_sud[1], sxt[0],
                                     start=True, stop=False)
                    nc.tensor.matmul(pSR[0][:], c_sud[3], sxt[1],
                                     start=False, stop=True)
                    # w3/w4 halves per h; one combined gpsimd add
                    w3 = spool.tile([128, 2 * FD], bf16, name="w3", tag="w3")
                    w4 = spool.tile([128, 2 * FD], bf16, name="w4", tag="w4")
                    x4t = spool.tile([128, 2 * FD], bf16, name="x4t",
                                     tag="x4t")
                    for h in range(2):
                        nc.vector.tensor_tensor(w3[:, h * FD:(h + 1) * FD],
                                                pSL[h][:], c_at3[0][h],
                                                Alu.mult)
                        nc.vector.tensor_tensor(w4[:, h * FD:(h + 1) * FD],
                                                pSR[h][:], c_at3[1][h],
                                                Alu.mult)
                    nc.gpsimd.tensor_tensor(x4t[:], w3[:], w4[:], Alu.add)
                    x4s = [x4t[:, 0:FD], x4t[:, FD:2 * FD]]

                # ---- stage E: temporal FC, PSUM-accumulated streams ----
                pH = [pp.tile([128, FD], f32, name="ph", tag=f"ph{F}", bufs=2)
                      for F in range(2)]
                if ty == 1:
                    streams = [(c_wq, sxa), (c_w2q, sxt)]
                elif ty == 3:
                    streams = [(c_wq, sxa), (c_wq, x4s)]
                else:
                    streams = [(c_wq, sxa)]
                ns = len(streams)
                for F in range(2):
                    for si, (w, s) in enumerate(streams):
                        for kh in range(2):
                            nc.tensor.matmul(
                                pH[F][:], w[kh][F], s[kh],
                                start=(si == 0 and kh == 0),
                                stop=(si == ns - 1 and kh == 1))

                # ---- stage F: out copies (ACT/DVE split) ----
                nc.scalar.copy(so_ap[:, 0:FD], pH[0][:])
                nc.vector.tensor_copy(so_ap[:, FD:2 * FD], pH[1][:])

            QG = 4                 # groups per input DMA
            OG = 2                 # groups per output DMA
            g = 0
            for ty in (0, 2, 1, 3):
                ngroups = Gs[ty]
                gi = 0
                while gi < ngroups:
                    nq = min(2 if g == 0 else QG, ngroups - gi)
                    sxg = xpool.tile([V, QG * NB * T], bf16, name="sxg",
                                     tag="sxg")
                    nc.gpsimd.dma_start(
                        sxg[:, 0:nq * NB * T].rearrange(
                            "v (g t) -> v g t", g=nq),
                        xg[g:g + nq].rearrange("g v t -> v g t"))
                    k = 0
                    while k < nq:
                        no = min(OG, nq - k)
                        so = spool.tile([128, OG * 2 * FD], bf16, name="so",
                                        tag="so", bufs=3)
                        for j in range(no):
                            emit_group(g, ty, sxg, (k + j) * NB * T,
                                       so[:, j * 2 * FD:(j + 1) * 2 * FD])
                            g += 1
                        nc.sync.dma_start(
                            yt[g - no:g].rearrange("g p w -> p g w"),
                            so[:, 0:no * 2 * FD].rearrange(
                                "p (g w) -> p g w", g=no))
                        k += no
                    gi += nq

    nc.compile()
    return nc


def _gate_types(x, mlp, if_make_dynamic, tau):
    """Exact replication of the reference gating; forward value is one-hot."""
    import jax
    import jax.numpy as jnp

    if not if_make_dynamic:
        return np.zeros(x.shape[0], dtype=np.int64)
    prob = jnp.asarray(x).mean(axis=1) @ jnp.asarray(mlp)
    u = jax.random.uniform(jax.random.key(42), prob.shape,
                           minval=1e-10, maxval=1.0)
    gumbel = -jnp.log(-jnp.log(u))
    soft = jax.nn.softmax((prob + gumbel) / tau, axis=-1)
    return np.asarray(jnp.argmax(soft, axis=-1), dtype=np.int64)


def kernel(x, mlp, adj_j, adj_t, adj_jc, adj_tj, fc_w, fc_b, alpha, beta,
           if_make_dynamic, tau):
    from concourse.bass_utils import run_bass_kernel_spmd

    x = np.asarray(x, dtype=np.float32)
    mlp = np.asarray(mlp, dtype=np.float32)
    adj_j = np.asarray(adj_j, dtype=np.float32)
    adj_t = np.asarray(adj_t, dtype=np.float32)
    adj_jc = np.asarray(adj_jc, dtype=np.float32)
    adj_tj = np.asarray(adj_tj, dtype=np.float32)
    fc_w = np.asarray(fc_w, dtype=np.float32)
    fc_b = np.asarray(fc_b, dtype=np.float32)
    alpha = np.asarray(alpha, dtype=np.float32).reshape(1, V, 1)
    beta = np.asarray(beta, dtype=np.float32).reshape(1, V, 1)

    types = _gate_types(x, mlp, if_make_dynamic, tau)
    counts = np.bincount(types, minlength=4)
    percore = N_CORES * NB
    Gs = tuple(int(np.ceil(c / percore)) for c in counts)
    G = sum(Gs)
    BLp = NB * G

    # per-core sample assignment: type-sorted, padded to uniform group counts
    order = np.argsort(types, kind="stable")
    perm = np.zeros((N_CORES, BLp), np.int64)
    real = np.zeros((N_CORES, BLp), bool)
    off = 0
    # group order in the program is (0, 2, 1, 3); slots must match
    slot_of_type = {}
    slot = 0
    for t in (0, 2, 1, 3):
        slot_of_type[t] = slot
        slot += NB * Gs[t]
    for t in range(4):
        n = int(counts[t])
        cap = NB * Gs[t]
        idx = order[off:off + n]
        off += n
        padded = np.zeros(N_CORES * cap, np.int64)
        padded[:n] = idx
        if N_CORES * cap > n and n > 0:
            padded[n:] = idx[0]
        rm = np.zeros(N_CORES * cap, bool)
        rm[:n] = True
        s = slot_of_type[t]
        perm[:, s:s + cap] = padded.reshape(N_CORES, cap)
        real[:, s:s + cap] = rm.reshape(N_CORES, cap)

    # ---- host-folded constants (two packed blobs) ----
    A1 = np.kron(adj_j, np.eye(3, dtype=np.float32))
    A3 = np.zeros((V, V), np.float32)
    for j in range(J):
        A3[3 * j:3 * j + 3, 3 * j:3 * j + 3] = adj_jc[j]
    I66 = np.eye(V, dtype=np.float32)

    idxs = np.arange(T)
    bandm = (np.abs(idxs[:, None] - idxs[None, :]) == 1).astype(np.float32)
    W2 = fc_w @ (adj_t * bandm)

    atj_lo = np.zeros((V, T), np.float32)
    atj_hi = np.zeros((V, T), np.float32)
    atj_lo[:, 1:] = adj_tj[:, np.arange(1, T), np.arange(0, T - 1)]
    atj_hi[:, :-1] = adj_tj[:, np.arange(0, T - 1), np.arange(1, T)]

    CB0 = 2 * 132 + 4 * 128
    CB1 = 8 * 128 + 4 * FD
    cb0 = np.zeros((128, CB0), np.float32)
    cb1 = np.zeros((128, CB1), np.float32)
    col = [0]

    def put(dst, arr):
        p, w = arr.shape
        dst[0:p, col[0]:col[0] + w] = arr
        col[0] += w

    put(cb0, np.concatenate([A1.T, I66], axis=1))
    put(cb0, np.concatenate([(A1 + A3).T, I66], axis=1))
    for kh in range(2):
        for F in range(2):
            put(cb0, fc_w[128 * F:128 * (F + 1), 128 * kh:128 * (kh + 1)].T)
    assert col[0] == CB0
    col[0] = 0
    for kh in range(2):
        for F in range(2):
            put(cb1, W2[128 * F:128 * (F + 1), 128 * kh:128 * (kh + 1)].T)
    sud0 = np.eye(128, k=1, dtype=np.float32)
    sud1 = np.eye(128, k=-1, dtype=np.float32)
    zs0 = np.zeros((128, 128), np.float32)
    zs0[127, 0] = 1.0               # SL h1 row0 = x[127] (from h0)
    zs1 = np.zeros((128, 128), np.float32)
    zs1[0, 127] = 1.0               # SR h0 row127 = x[128] (from h1)
    put(cb1, sud0)
    put(cb1, sud1)
    put(cb1, zs0)
    put(cb1, zs1)
    for src in (atj_lo, atj_hi):
        for h in range(2):
            put(cb1, np.tile(src[:, 128 * h:128 * (h + 1)].T, (1, NB)))
    assert col[0] == CB1
    cb0 = cb0.astype(BF16)
    cb1 = cb1.astype(BF16)

    x_bf = x.astype(BF16)
    in_maps = []
    for c in range(N_CORES):
        xp = x_bf[perm[c]]                                     # [BLp, V, T]
        xgc = np.ascontiguousarray(
            xp.reshape(G, NB, V, T).transpose(0, 2, 1, 3)
        ).reshape(G, V, NB * T)
        in_maps.append(dict(xg=xgc, cb0=cb0, cb1=cb1))

    # program group order is types (0, 2, 1, 3); Gs passed in type order but
    # _build_nc iterates (0, 2, 1, 3) so slot layout matches perm layout.
    if Gs not in _CACHE:
        _CACHE[Gs] = _build_nc(Gs)
    nc = _CACHE[Gs]
    res = run_bass_kernel_spmd(nc, in_maps, core_ids=list(range(N_CORES)),
                               **_RUN_KW)
    _LAST_RES.clear()
    _LAST_RES["res"] = res

    # ---- host epilogue: un-transpose, + fc_b, LN, alpha/beta, residual ----
    out = np.empty((B, V, T), dtype=np.float32)
    for c in range(N_CORES):
        H = np.asarray(res.results[c]["yt"], dtype=np.float32)
        # yt: [G, 128(p), 2(F), NB, V] -> H[b, f=F*128+p, v]
        H = (H.reshape(G, 128, 2, NB, V).transpose(0, 3, 2, 1, 4)
             .reshape(BLp, T, V).transpose(0, 2, 1))           # [b, V, T(f)]
        H += fc_b[None, None, :]
        m = H.mean(axis=1, keepdims=True)
        var = ((H - m) ** 2).mean(axis=1, keepdims=True)
        h = (H - m) / np.sqrt(var + 1e-5) * alpha + beta
        res_c = x[perm[c]] + h
        msk = real[c]
        out[perm[c][msk]] = res_c[msk]
    return out


_RUN_KW = {}
_LAST_RES = {}
